# revision 21
# baseline (speedup 1.0000x reference)
"""Bass/Trainium2 kernel for nn_EquivariantReynoldsWrap.

The reference module is linear in x: for every pixel,
    out = (1/G) * sum_g BlockDiag(A_g) @ Wf @ BlockDiag(Ainv_g) @ x_pixel
so the whole pipeline collapses into one 64x64 channel-mixing matrix M,
computed on host (cheap). The device work is a single 1x1-conv matmul
out[b] = M @ x[b] with x[b] viewed as (64, H*W).

Sharding: data-parallel over B across the 8 cores (1 batch each).
Per core the two halves of the pixel axis are interleaved on the
partition axis (partition p = channel p//2, half p%2) and the stationary
weight is the 128x128 interleaved block-diagonal of M^T, so each
512-column matmul covers 1024 pixels.

I/O in bf16 (half the DMA bytes of f32; the 2e-2 accuracy budget is
~10x above bf16's ~2e-3). The kernel is latency-bound at the tail:
after the last input chunk lands, it still pays DMA-sem propagation
(~0.9us) + PSUM->SBUF copy + DMA trigger (~0.6us SEQ) + DGE start
delay (~0.8us) + the last store. So:
  - each matmul is followed by a cheap 128-col guard matmul that covers
    the systolic drain, so chunk i's copy starts right after matmul i
    instead of waiting for the (data-gated) matmul i+1;
  - copies are split DVE (cols 0:256) + GPSIMD (256:512) so the
    Activation engine only runs DMA triggers and a copy never queues
    behind a ~0.6us trigger;
  - trigger load: sync {w, x1, x2, y0, y2}, scalar {x0, x3, y1, y3}.

Raw bacc (no TileContext): hand-rolled semaphores, minimal head/tail.
"""

import numpy as np
import ml_dtypes

import concourse.bacc as bacc
import concourse.bass as bass
from concourse import mybir
from concourse.bass_utils import run_bass_kernel_spmd

B, C, H, W_SP = 8, 64, 64, 64
COUT = 64
HW = H * W_SP          # 4096 pixels per batch
HALF = HW // 2         # 2048 -> stacked column count per core
N_CORES = 8

# chunk taper [512, 512, 512, 256, 256]: the matmul chain's end is
# PE-bound (2048 columns total) regardless of chunking, so the LAST two
# chunks in the matmul queue are small -- their PSUM->SBUF copy and
# store stages (the post-chain tail) halve. Each chunk owns whole PSUM
# banks exclusively (concurrent same-bank access from two engines
# wedges the device); the final 256-col chunk sits alone in bank 4.
XCS = [(0, 512), (512, 1024), (1024, 1536), (1536, 1792), (1792, 2048)]
PCS = [(0, 512), (512, 1024), (1024, 1536), (1536, 1792), (2048, 2304)]
N_CHUNKS = 5
PSUM_COLS = 2560
N_WARM = 4             # bf16 warm-up matmuls (PE clock ramp)

TRACE = False          # test.py flips this to profile
_cached_nc = None

BF16 = ml_dtypes.bfloat16


def _build_nc():
    global _cached_nc
    if _cached_nc is not None:
        return _cached_nc

    bf16 = mybir.dt.bfloat16
    f32 = mybir.dt.float32

    nc = bacc.Bacc(
        "TRN2",
        target_bir_lowering=False,
        debug=False,
        enable_asserts=False,
        num_devices=N_CORES,
    )
    xd = nc.dram_tensor("x", [C, HW], bf16, kind="ExternalInput").ap()
    wd = nc.dram_tensor("w", [128, 128], bf16, kind="ExternalInput").ap()
    yd = nc.dram_tensor("y", [COUT, HW], bf16, kind="ExternalOutput").ap()

    # [64, 2, t] c-major outer dims: the DMA pairs partition p with
    # (c=p//2, s=p%2); the outer dim of 64 spreads each transfer across
    # all 16 SDMA engines (an outer dim of 2 used only 2 of them).
    xr = xd.rearrange("c (s t) -> c s t", s=2)
    yr = yd.rearrange("c (s t) -> c s t", s=2)

    with (
        nc.sbuf_tensor("wt", [128, 128], bf16) as wt_t,
        nc.sbuf_tensor("xt", [128, HALF], bf16) as xt_t,
        nc.sbuf_tensor("ot", [128, HALF], bf16) as ot_t,
        nc.sbuf_tensor("zt", [128, 512], mybir.dt.bfloat16) as zt_t,
        nc.psum_tensor([128, PSUM_COLS], f32) as ps_t,
        nc.psum_tensor([128, 512], f32) as wps_t,
        nc.semaphore("s_w") as s_w,
        nc.semaphore("s_x0") as s_x0,
        nc.semaphore("s_x1") as s_x1,
        nc.semaphore("s_x2") as s_x2,
        nc.semaphore("s_x3") as s_x3,
        nc.semaphore("s_x4") as s_x4,
        nc.semaphore("s_mm") as s_mm,
        nc.semaphore("s_c0") as s_c0,
        nc.semaphore("s_c1") as s_c1,
        nc.semaphore("s_c2") as s_c2,
        nc.semaphore("s_c3") as s_c3,
        nc.semaphore("s_c4") as s_c4,
        nc.semaphore("s_y") as s_y,
    ):
        wt = wt_t.ap()
        xt = xt_t.ap()
        ot = ot_t.ap()
        zt = zt_t.ap()
        ps = ps_t.ap()
        wps = wps_t.ap()

        def xs_(i):
            return slice(*XCS[i])

        def ps_(i):
            return slice(*PCS[i])

        sync, scalar, tensor, vector, gpsimd = (
            nc.sync, nc.scalar, nc.tensor, nc.vector, nc.gpsimd
        )

        # input triggers: w alone-first on the pool ring (its completion
        # sem on a shared HW ring lands only after ALL later transfers on
        # that ring); pool also takes one small chunk. HW rings carry the
        # rest, small chunks last.
        gpsimd.dma_start(wt[:], wd[:]).then_inc(s_w, 16)
        gpsimd.dma_start(xt[:, xs_(4)], xr[:, :, xs_(4)]).then_inc(s_x4, 16)
        sync.dma_start(xt[:, xs_(0)], xr[:, :, xs_(0)]).then_inc(s_x0, 16)
        sync.dma_start(xt[:, xs_(2)], xr[:, :, xs_(2)]).then_inc(s_x2, 16)
        scalar.dma_start(xt[:, xs_(1)], xr[:, :, xs_(1)]).then_inc(s_x1, 16)
        scalar.dma_start(xt[:, xs_(3)], xr[:, :, xs_(3)]).then_inc(s_x3, 16)

        # warm-up matmuls on the (uninitialized) zt tile: results go to
        # wps which is never read, so garbage inputs are fine.
        for _ in range(N_WARM):
            tensor.matmul(wps[:], zt[:, :128], zt[:])

        # chunk matmuls; copies gate on each matmul's own retire-inc (the
        # copy engines' slower column rate never catches the ~128-column
        # systolic drain, so no guard matmul is needed).
        tensor.wait_ge(s_w, 16)
        xsem = [s_x0, s_x1, s_x2, s_x3, s_x4]
        for i in range(N_CHUNKS):
            tensor.wait_ge(xsem[i], 16)
            tensor.matmul(ps[:, ps_(i)], wt[:], xt[:, xs_(i)]).then_inc(s_mm)

        # copies (cast f32 PSUM -> bf16 SBUF): DVE 0,2,4; ACT 1,3.
        vector.wait_ge(s_mm, 1)
        vector.tensor_copy(ot[:, xs_(0)], ps[:, ps_(0)]).then_inc(s_c0)
        vector.wait_ge(s_mm, 3)
        vector.tensor_copy(ot[:, xs_(2)], ps[:, ps_(2)]).then_inc(s_c2)
        vector.wait_ge(s_mm, 5)
        vector.tensor_copy(ot[:, xs_(4)], ps[:, ps_(4)]).then_inc(s_c4)

        scalar.wait_ge(s_mm, 2)
        scalar.copy(ot[:, xs_(1)], ps[:, ps_(1)]).then_inc(s_c1)
        scalar.wait_ge(s_c1, 1)
        scalar.dma_start(yr[:, :, xs_(1)], ot[:, xs_(1)]).then_inc(s_y, 16)
        scalar.wait_ge(s_mm, 4)
        scalar.copy(ot[:, xs_(3)], ps[:, ps_(3)]).then_inc(s_c3)
        scalar.wait_ge(s_c3, 1)
        scalar.dma_start(yr[:, :, xs_(3)], ot[:, xs_(3)]).then_inc(s_y, 16)

        sync.wait_ge(s_c0, 1)
        sync.dma_start(yr[:, :, xs_(0)], ot[:, xs_(0)]).then_inc(s_y, 16)
        sync.wait_ge(s_c2, 1)
        sync.dma_start(yr[:, :, xs_(2)], ot[:, xs_(2)]).then_inc(s_y, 16)
        sync.wait_ge(s_c4, 1)
        sync.dma_start(yr[:, :, xs_(4)], ot[:, xs_(4)]).then_inc(s_y, 16)
        _ = s_y

    nc.compile()
    _cached_nc = nc
    return nc


def _fuse_weights(group_tensor, group_tensor_inv, Wf):
    A = np.asarray(group_tensor, np.float64)
    Ai = np.asarray(group_tensor_inv, np.float64)
    Wf64 = np.asarray(Wf, np.float64)
    G, CG, _ = A.shape
    n = C // CG
    eye = np.eye(n)
    M = np.zeros((COUT, C))
    for g in range(G):
        M += np.kron(eye, A[g]) @ Wf64 @ np.kron(eye, Ai[g])
    M /= G
    MT = np.ascontiguousarray(M.T).astype(np.float32)
    # interleaved packing: x-tile partition p holds channel p//2 of pixel
    # half p%2; out partition q holds channel q//2 of half q%2.
    W2T = np.zeros((128, 128), np.float32)
    W2T[0::2, 0::2] = MT
    W2T[1::2, 1::2] = MT
    return W2T.astype(BF16)


def kernel(x, group_tensor, group_tensor_inv, Wf):
    nc = _build_nc()
    W2T = _fuse_weights(group_tensor, group_tensor_inv, Wf)
    x = np.ascontiguousarray(np.asarray(x, np.float32).astype(BF16))

    in_maps = [
        {"x": x[b].reshape(C, HW), "w": W2T} for b in range(B)
    ]
    res = run_bass_kernel_spmd(
        nc, in_maps, core_ids=list(range(N_CORES)), trace=TRACE
    )
    if TRACE:
        kernel.last_results = res
    y = np.stack(
        [
            res.results[b]["y"].astype(np.float32).reshape(COUT, H, W_SP)
            for b in range(B)
        ]
    )
    return y


# revision 22
# speedup vs baseline: 1.0500x; 1.0500x over previous
"""Bass/Trainium2 kernel for nn_EquivariantReynoldsWrap.

The reference module is linear in x: for every pixel,
    out = (1/G) * sum_g BlockDiag(A_g) @ Wf @ BlockDiag(Ainv_g) @ x_pixel
so the whole pipeline collapses into one 64x64 channel-mixing matrix M,
computed on host (cheap). The device work is a single 1x1-conv matmul
out[b] = M @ x[b] with x[b] viewed as (64, H*W).

Sharding: data-parallel over B across the 8 cores (1 batch each).
Per core the two halves of the pixel axis are interleaved on the
partition axis (partition p = channel p//2, half p%2) and the stationary
weight is the 128x128 interleaved block-diagonal of M^T, so each
512-column matmul covers 1024 pixels.

I/O in bf16 (half the DMA bytes of f32; the 2e-2 accuracy budget is
~10x above bf16's ~2e-3). The kernel is latency-bound at the tail:
after the last input chunk lands, it still pays DMA-sem propagation
(~0.9us) + PSUM->SBUF copy + DMA trigger (~0.6us SEQ) + DGE start
delay (~0.8us) + the last store. So:
  - each matmul is followed by a cheap 128-col guard matmul that covers
    the systolic drain, so chunk i's copy starts right after matmul i
    instead of waiting for the (data-gated) matmul i+1;
  - copies are split DVE (cols 0:256) + GPSIMD (256:512) so the
    Activation engine only runs DMA triggers and a copy never queues
    behind a ~0.6us trigger;
  - trigger load: sync {w, x1, x2, y0, y2}, scalar {x0, x3, y1, y3}.

Raw bacc (no TileContext): hand-rolled semaphores, minimal head/tail.
"""

import numpy as np
import ml_dtypes

import concourse.bacc as bacc
import concourse.bass as bass
from concourse import mybir
from concourse.bass_utils import run_bass_kernel_spmd

B, C, H, W_SP = 8, 64, 64, 64
COUT = 64
HW = H * W_SP          # 4096 pixels per batch
HALF = HW // 2         # 2048 -> stacked column count per core
N_CORES = 8

CH = 512               # columns per pipeline chunk
N_CHUNKS = HALF // CH  # 4
HC = CH // 2           # copy split point within a chunk
N_WARM = 4             # bf16 warm-up matmuls (HAM un-throttle)

TRACE = False          # test.py flips this to profile
_cached_nc = None

BF16 = ml_dtypes.bfloat16


def _build_nc():
    global _cached_nc
    if _cached_nc is not None:
        return _cached_nc

    bf16 = mybir.dt.bfloat16
    f32 = mybir.dt.float32

    nc = bacc.Bacc(
        "TRN2",
        target_bir_lowering=False,
        debug=False,
        enable_asserts=False,
        num_devices=N_CORES,
    )
    xd = nc.dram_tensor("x", [C, HW], bf16, kind="ExternalInput").ap()
    wd = nc.dram_tensor("w", [128, 128], bf16, kind="ExternalInput").ap()
    yd = nc.dram_tensor("y", [COUT, HW], bf16, kind="ExternalOutput").ap()

    # [64, 2, t] c-major outer dims: the DMA pairs partition p with
    # (c=p//2, s=p%2); the outer dim of 64 spreads each transfer across
    # all 16 SDMA engines (an outer dim of 2 used only 2 of them).
    xr = xd.rearrange("c (s t) -> c s t", s=2)
    yr = yd.rearrange("c (s t) -> c s t", s=2)

    with (
        nc.sbuf_tensor("wt", [128, 128], bf16) as wt_t,
        nc.sbuf_tensor("xt", [128, HALF], bf16) as xt_t,
        nc.sbuf_tensor("ot", [128, HALF], bf16) as ot_t,
        nc.sbuf_tensor("zt", [128, 512], mybir.dt.bfloat16) as zt_t,
        nc.psum_tensor([128, HALF], f32) as ps_t,
        nc.psum_tensor([128, 512], f32) as wps_t,
        nc.semaphore("s_w") as s_w,      # weights DMA done
        # one sem per x-chunk DMA: a sem shared by two DMAs on one ring
        # reaches 16 from a MIX of the two transfers' per-engine incs
        nc.semaphore("s_x0") as s_x0,
        nc.semaphore("s_x1") as s_x1,
        nc.semaphore("s_x2") as s_x2,
        nc.semaphore("s_x3") as s_x3,
        nc.semaphore("s_z") as s_z,      # warmup tile zeroed
        nc.semaphore("s_mm") as s_mm,    # matmul+guard pairs (2 per chunk)
        nc.semaphore("s_c0") as s_c0,    # chunk copy done (2 halves)
        nc.semaphore("s_c1") as s_c1,
        nc.semaphore("s_c2") as s_c2,
        nc.semaphore("s_c3") as s_c3,
        nc.semaphore("s_y") as s_y,      # out DMAs
    ):
        wt = wt_t.ap()
        xt = xt_t.ap()
        ot = ot_t.ap()
        zt = zt_t.ap()
        ps = ps_t.ap()
        wps = wps_t.ap()

        def cs(i):
            return slice(i * CH, (i + 1) * CH)

        def csl(i):  # low copy half
            return slice(i * CH, i * CH + HC)

        def csh(i):  # high copy half
            return slice(i * CH + HC, (i + 1) * CH)

        # Linear emission into the entry basic block (no nc.Block): avoids
        # the per-engine body branches (I$ misses) and the Block exit
        # barrier; the walrus-generated NEFF epilogue handles quiescence
        # and zeroes all semaphores for re-execution.
        sync, scalar, tensor, vector, gpsimd = (
            nc.sync, nc.scalar, nc.tensor, nc.vector, nc.gpsimd
        )

        # ring assignment by completion order: each ring's DMAs complete
        # serially, so chunk i's gate is matched to the i-th completing
        # transfer across the two rings:
        #   scalar 1st -> c0, sync 2nd (after small w) -> c1,
        #   sync 3rd -> c2, scalar 2nd -> c3
        gpsimd.dma_start(wt[:], wd[:]).then_inc(s_w, 16)
        gpsimd.dma_start(xt[:, cs(3)], xr[:, :, cs(3)]).then_inc(s_x3, 16)
        sync.dma_start(xt[:, cs(0)], xr[:, :, cs(0)]).then_inc(s_x0, 16)
        sync.dma_start(xt[:, cs(2)], xr[:, :, cs(2)]).then_inc(s_x2, 16)
        scalar.dma_start(xt[:, cs(1)], xr[:, :, cs(1)]).then_inc(s_x1, 16)

        # HAM warm-up matmuls on the (uninitialized) zt tile: results go
        # to wps which is never read, so garbage inputs are fine. No gate:
        # they start at tensor's preamble end and ramp the PE clock.
        for _ in range(N_WARM):
            tensor.matmul(wps[:], zt[:, :128], zt[:])

        # A matmul's sem update fires at instruction retire (last column
        # ENTERS the array); the ~128-cycle systolic drain is still
        # writing PSUM then. Each chunk matmul is followed by a short
        # guard matmul on the zero tile whose retire covers the drain, so
        # chunk i's copies gate on s_mm >= 2i+2 without waiting for the
        # data-gated matmul i+1.
        tensor.wait_ge(s_w, 16)
        xs = [s_x0, s_x1, s_x2, s_x3]
        for i in range(N_CHUNKS):
            tensor.wait_ge(xs[i], 16)
            tensor.matmul(ps[:, cs(i)], wt[:], xt[:, cs(i)]).then_inc(s_mm)

        # copies (cast f32 PSUM -> bf16 SBUF): DVE takes chunks 0, 2; ACT
        # takes 1, 3.
        vector.wait_ge(s_mm, 1)
        vector.tensor_copy(ot[:, cs(0)], ps[:, cs(0)]).then_inc(s_c0)
        vector.wait_ge(s_mm, 3)
        vector.tensor_copy(ot[:, cs(2)], ps[:, cs(2)]).then_inc(s_c2)

        scalar.wait_ge(s_mm, 2)
        scalar.copy(ot[:, cs(1)], ps[:, cs(1)]).then_inc(s_c1)
        scalar.wait_ge(s_mm, 4)
        scalar.copy(ot[:, cs(3)], ps[:, cs(3)]).then_inc(s_c3)
        scalar.wait_ge(s_c3, 1)
        scalar.dma_start(yr[:, :, cs(3)], ot[:, cs(3)]).then_inc(s_y, 16)

        sync.wait_ge(s_c0, 1)
        sync.dma_start(yr[:, :, cs(0)], ot[:, cs(0)]).then_inc(s_y, 16)
        sync.wait_ge(s_c1, 1)
        sync.dma_start(yr[:, :, cs(1)], ot[:, cs(1)]).then_inc(s_y, 16)
        sync.wait_ge(s_c2, 1)
        sync.dma_start(yr[:, :, cs(2)], ot[:, cs(2)]).then_inc(s_y, 16)
        # the NEFF epilogue's per-ring DGE drains hold teardown until all
        # output descriptors (data + sem incs) have retired
        _ = s_y

    nc.compile()
    _cached_nc = nc
    return nc


def _fuse_weights(group_tensor, group_tensor_inv, Wf):
    A = np.asarray(group_tensor, np.float64)
    Ai = np.asarray(group_tensor_inv, np.float64)
    Wf64 = np.asarray(Wf, np.float64)
    G, CG, _ = A.shape
    n = C // CG
    eye = np.eye(n)
    M = np.zeros((COUT, C))
    for g in range(G):
        M += np.kron(eye, A[g]) @ Wf64 @ np.kron(eye, Ai[g])
    M /= G
    MT = np.ascontiguousarray(M.T).astype(np.float32)
    # interleaved packing: x-tile partition p holds channel p//2 of pixel
    # half p%2; out partition q holds channel q//2 of half q%2.
    W2T = np.zeros((128, 128), np.float32)
    W2T[0::2, 0::2] = MT
    W2T[1::2, 1::2] = MT
    return W2T.astype(BF16)


def kernel(x, group_tensor, group_tensor_inv, Wf):
    nc = _build_nc()
    W2T = _fuse_weights(group_tensor, group_tensor_inv, Wf)
    x = np.ascontiguousarray(np.asarray(x, np.float32).astype(BF16))

    in_maps = [
        {"x": x[b].reshape(C, HW), "w": W2T} for b in range(B)
    ]
    res = run_bass_kernel_spmd(
        nc, in_maps, core_ids=list(range(N_CORES)), trace=TRACE
    )
    if TRACE:
        kernel.last_results = res
    y = np.stack(
        [
            res.results[b]["y"].astype(np.float32).reshape(COUT, H, W_SP)
            for b in range(B)
        ]
    )
    return y
